# revision 1
# baseline (speedup 1.0000x reference)
"""Sliding-window causal attention (B=1, S=4096, E=1024, H=16, D=64,
window=(256,0)) on 8 TRN2 NeuronCores.

Sharding: pure sequence-parallel. Core c computes queries [512c, 512c+512)
and needs keys [512c-256, 512c+512) — a 256-row halo. No collectives.

Layout: everything transposed ([e, s]) so QKV projections, the RoPE
rotation, scores, PV and the output projection are all TensorE matmuls.
Matmul inputs are bf16 (1 cyc/row on PE); accumulation is f32 in PSUM.

Per 128-query tile the 256-left/causal window spans exactly 3 key tiles
of 128 (band structure identical for every tile in the padded local
frame). Window+halo masking is one additive bias input fused before the
ACT exp; probs are normalized before the PV transpose so nothing
downstream needs per-head softmax sums.
"""

import os
import sys

sys.path.insert(0, "/opt/trn_rl_repo")

import math

import numpy as np
import ml_dtypes

import concourse.bass as bass
from concourse import bacc
import concourse.mybir as mybir
import concourse.tile as tile
from concourse.bass_utils import run_bass_kernel_spmd


def _ensure_ntff_hook():
    """Register the axon NTFF profile hook so trace=True works.

    The image's antenv package lacks axon_hooks; provide one and wire it
    to trn_agent_boot's ctypes profiler. No-op when unavailable.
    """
    import types
    try:
        import antenv
    except ImportError:
        return
    if "antenv.axon_hooks" not in sys.modules:
        mod = types.ModuleType("antenv.axon_hooks")
        mod._hook = None
        def _set(h, _m=mod):
            _m._hook = h
        def _get(_m=mod):
            return _m._hook
        mod.set_axon_ntff_profile_hook = _set
        mod.get_axon_ntff_profile_hook = _get
        sys.modules["antenv.axon_hooks"] = mod
        antenv.axon_hooks = mod
    hooks = sys.modules["antenv.axon_hooks"]
    if hooks.get_axon_ntff_profile_hook() is None:
        try:
            from trn_agent_boot.trn_boot import _ntff_profile_via_ctypes
            hooks.set_axon_ntff_profile_hook(
                _ntff_profile_via_ctypes("/opt/axon/libaxon_pjrt.so"))
        except Exception:
            pass

BF16 = mybir.dt.bfloat16
F32 = mybir.dt.float32

NCORES = 8
S = 4096
E = 1024
H = 16
D = 64
SL = S // NCORES        # 512 local queries per core
HALO = 256
SK = SL + HALO          # 768 local keys (padded frame)
NQT = SL // 128         # 4 query tiles
NKT = SK // 128         # 6 key tiles
NET = E // 128          # 8 embed tiles
SCALE = 1.0 / math.sqrt(D)
NEG = -3.0e10           # additive mask; * SCALE still << f32 exp underflow


def _build_graph():
    nc = bacc.Bacc("TRN2", target_bir_lowering=False, debug=False, num_devices=NCORES)

    # ---- DRAM parameters (per-core shards staged by kernel()) ----
    xT = nc.declare_dram_parameter("xT", [E, SK], BF16, isOutput=False)
    wq = nc.declare_dram_parameter("wq", [E, E], BF16, isOutput=False)
    wk = nc.declare_dram_parameter("wk", [E, E], BF16, isOutput=False)
    wv = nc.declare_dram_parameter("wv", [E, E], BF16, isOutput=False)
    wo = nc.declare_dram_parameter("wo", [E, E], BF16, isOutput=False)
    rt = nc.declare_dram_parameter("rt", [128, 128], BF16, isOutput=False)
    bq = nc.declare_dram_parameter("bq", [128, NET], F32, isOutput=False)
    bo = nc.declare_dram_parameter("bo", [128, NET], F32, isOutput=False)
    bv = nc.declare_dram_parameter("bv", [128, E], F32, isOutput=False)
    cosT = nc.declare_dram_parameter("cosT", [128, SK], F32, isOutput=False)
    sinT = nc.declare_dram_parameter("sinT", [128, SK], F32, isOutput=False)
    maskadd = nc.declare_dram_parameter("maskadd", [128, NKT, 512], BF16, isOutput=False)
    out_ext = nc.declare_dram_parameter("out", [E, SL], F32, isOutput=True)

    with tile.TileContext(nc) as tc:
        with (
            # weights: 24 slots of [128,1024] bf16 (wq+wk+wv); wo reuses slots
            tc.tile_pool(name="wpool", bufs=1) as wpool,
            tc.tile_pool(name="xpool", bufs=1) as xpool,
            tc.tile_pool(name="qk", bufs=1) as qkpool,
            tc.tile_pool(name="vpool", bufs=1) as vpool,
            tc.tile_pool(name="cs", bufs=1) as cspool,
            tc.tile_pool(name="small", bufs=1) as small,
            tc.tile_pool(name="rope", bufs=2) as ropepool,
            tc.tile_pool(name="att", bufs=3) as attpool,
            tc.tile_pool(name="ptf", bufs=2) as ptpool,
            tc.tile_pool(name="ctx", bufs=1) as ctxpool,
            tc.tile_pool(name="outp", bufs=2) as outpool,
            tc.tile_pool(name="mm", bufs=4, space="PSUM") as mmps,
            tc.tile_pool(name="sc", bufs=2, space="PSUM") as scps,
            tc.tile_pool(name="cx", bufs=2, space="PSUM") as cxps,
        ):
            # ---------- load constants ----------
            rt_sb = small.tile([128, 128], BF16, tag="rt")
            nc.sync.dma_start(rt_sb[:], rt[:])
            bq_sb = small.tile([128, NET], F32, tag="bq")
            nc.sync.dma_start(bq_sb[:], bq[:])
            bo_sb = small.tile([128, NET], F32, tag="bo")
            nc.sync.dma_start(bo_sb[:], bo[:])
            bv_sb = small.tile([128, E], F32, tag="bv")
            nc.sync.dma_start(bv_sb[:], bv[:])
            cos_sb = cspool.tile([128, SK], F32, tag="cos")
            nc.sync.dma_start(cos_sb[:], cosT[:])
            sin_sb = cspool.tile([128, SK], F32, tag="sin")
            nc.sync.dma_start(sin_sb[:], sinT[:])
            mask_sb = cspool.tile([128, NKT, 512], BF16, tag="mask")
            nc.sync.dma_start(mask_sb[:], maskadd[:])

            x_sb = []
            for kt in range(NET):
                t = xpool.tile([128, SK], BF16, tag=f"x{kt}")
                nc.sync.dma_start(t[:], xT[kt * 128 : (kt + 1) * 128, :])
                x_sb.append(t)

            w_sb = {}
            for name, ext in (("q", wq), ("k", wk), ("v", wv)):
                tiles = []
                for kt in range(NET):
                    t = wpool.tile([128, E], BF16, tag=f"w{name}{kt}")
                    nc.sync.dma_start(t[:], ext[kt * 128 : (kt + 1) * 128, :])
                    tiles.append(t)
                w_sb[name] = tiles

            # ---------- QKV projections + RoPE ----------
            # qT[e, s] = Wq.T @ xT  (lhsT = Wq tile [ke, e])
            q_rope = []   # 8 tiles [128, SL] bf16  (rows = 2 heads x 64)
            k_rope = []   # 8 tiles [128, SK] bf16
            v_sb = []     # 6 tiles [128, E] bf16   (rows = local seq)

            def project(which, et, n0, n1, psum):
                for kt in range(NET):
                    nc.tensor.matmul(
                        psum[:, 0 : n1 - n0],
                        w_sb[which][kt][:, et * 128 : (et + 1) * 128],
                        x_sb[kt][:, n0:n1],
                        start=(kt == 0),
                        stop=(kt == NET - 1),
                    )

            # Two-stage pipeline: projections of tile et are issued before
            # the RoPE rotation matmuls of tile et-1, so the PE's in-order
            # queue never stalls on the ACT bias/copy step.
            def emit_rope(q_lin, k_lin, et):
                rotp = mmps.tile([128, 512], F32, tag="mm", name="rotp")
                nc.tensor.matmul(rotp[:, 0:SL], rt_sb[:], q_lin[:],
                                 start=True, stop=True)
                rotk = mmps.tile([128, 512], F32, tag="mm", name="rotk")
                rotk2 = mmps.tile([128, 512], F32, tag="mm", name="rotk2")
                nc.tensor.matmul(rotk[:, :], rt_sb[:], k_lin[:, 0:512],
                                 start=True, stop=True)
                nc.tensor.matmul(rotk2[:, 0 : SK - 512], rt_sb[:],
                                 k_lin[:, 512:SK], start=True, stop=True)
                t1 = ropepool.tile([128, SL], F32, tag="t1", name="t1")
                nc.vector.tensor_mul(t1[:], q_lin[:], cos_sb[:, HALO:SK])
                t2 = ropepool.tile([128, SL], F32, tag="t2", name="t2")
                nc.vector.tensor_mul(t2[:], rotp[:, 0:SL], sin_sb[:, HALO:SK])
                qf = qkpool.tile([128, SL], BF16, tag=f"qf{et}", name=f"qf{et}")
                nc.vector.tensor_add(qf[:], t1[:], t2[:])
                q_rope.append(qf)
                t3 = ropepool.tile([128, SK], F32, tag="t3", name="t3")
                nc.vector.tensor_mul(t3[:], k_lin[:], cos_sb[:])
                t4 = ropepool.tile([128, SK], F32, tag="t4", name="t4")
                nc.vector.tensor_mul(t4[:, 0:512], rotk[:, :], sin_sb[:, 0:512])
                nc.vector.tensor_mul(t4[:, 512:SK], rotk2[:, 0 : SK - 512],
                                     sin_sb[:, 512:SK])
                kf = qkpool.tile([128, SK], BF16, tag=f"kf{et}", name=f"kf{et}")
                nc.vector.tensor_add(kf[:], t3[:], t4[:])
                k_rope.append(kf)

            pending = None
            for et in range(NET):
                # ---- q: only real rows (cols HALO..SK of padded frame) ----
                qp = mmps.tile([128, 512], F32, tag="mm")
                project("q", et, HALO, SK, qp)
                q_lin = ropepool.tile([128, SL], BF16, tag="qlin")
                # +bq (per-partition bias in transposed layout) and cast bf16
                nc.scalar.activation(
                    q_lin[:], qp[:, 0:SL],
                    mybir.ActivationFunctionType.Identity,
                    bias=bq_sb[:, et : et + 1], scale=1.0,
                )
                # ---- k: all SK rows, no bias ----
                kp = mmps.tile([128, 512], F32, tag="mm")
                project("k", et, 0, 512, kp)
                kp2 = mmps.tile([128, 512], F32, tag="mm")
                project("k", et, 512, SK, kp2)
                k_lin = ropepool.tile([128, SK], BF16, tag="klin")
                nc.scalar.copy(k_lin[:, 0:512], kp[:, 0:512])
                nc.scalar.copy(k_lin[:, 512:SK], kp2[:, 0 : SK - 512])
                if pending is not None:
                    emit_rope(*pending)
                pending = (q_lin, k_lin, et)
            emit_rope(*pending)

            # ---- v: natural layout [s, e] with a ones column per head ----
            # v_aug[st] is [128, 16*65]: per head 64 value dims + 1 ones
            # column, so the PV matmul's M=65th row accumulates the
            # softmax denominator for free.
            for st in range(NKT):
                vt = vpool.tile([128, 16 * 65], BF16, tag=f"v{st}")
                nc.gpsimd.memset(
                    vt[:].rearrange("p (h c) -> p h c", c=65)[:, :, 64:65], 1.0
                )
                for half in range(2):
                    vp = mmps.tile([128, 512], F32, tag="mm")
                    for kt in range(NET):
                        nc.tensor.matmul(
                            vp[:],
                            x_sb[kt][:, st * 128 : (st + 1) * 128],
                            w_sb["v"][kt][:, half * 512 : (half + 1) * 512],
                            start=(kt == 0),
                            stop=(kt == NET - 1),
                        )
                    # +bv (varies along free dim), cast bf16, 65-stride heads
                    dst = vt[:, half * 8 * 65 : (half * 8 + 8) * 65].rearrange(
                        "p (h c) -> p h c", c=65
                    )[:, :, 0:64]
                    nc.vector.tensor_add(
                        dst, vp[:].rearrange("p (h c) -> p h c", c=64),
                        bv_sb[:, half * 512 : (half + 1) * 512].rearrange(
                            "p (h c) -> p h c", c=64
                        ),
                    )
                v_sb.append(vt)

            # ---------- attention (scores computed transposed) ----------
            ctx_sb = [ctxpool.tile([128, SL], BF16, tag=f"ctx{et}", name=f"ctx{et}")
                      for et in range(NET)]

            # q-column range covered by k-tile kt (band: t <= kt <= t+2)
            def qrange(kt):
                lo = max(0, kt - 2) * 128
                hi = (min(NQT - 1, kt) + 1) * 128
                return lo, hi

            pt = {}

            def emit_scores(hp):
                # interleave the head pair: their lhsT live in partition
                # rows 0-63 / 64-127 -> distinct PE row groups, so the
                # matmuls overlap on the array.
                et = hp
                for kt in range(NKT):
                    lo, hi = qrange(kt)
                    for sub in (0, 64):
                        h = 2 * hp + (sub // 64)
                        sp = scps.tile([128, 384], F32, tag="sc",
                                       name=f"sc{h}_{kt}")
                        nc.tensor.matmul(
                            sp[:, 0 : hi - lo],
                            k_rope[et][sub : sub + 64,
                                       kt * 128 : (kt + 1) * 128],
                            q_rope[et][sub : sub + 64, lo:hi],
                            start=True, stop=True,
                        )
                        pe = ptpool.tile([128, SL], BF16, tag="pe",
                                         name=f"pe{kt}_{sub}", bufs=4)
                        nc.scalar.activation(
                            pe[:, lo:hi], sp[:, 0 : hi - lo],
                            mybir.ActivationFunctionType.Exp,
                            bias=0.0, scale=SCALE,
                        )
                        pm = ptpool.tile([128, SL], BF16, tag=f"pm{kt}_{sub}",
                                         name=f"pm{kt}_{sub}")
                        nc.vector.tensor_tensor(
                            pm[:, lo:hi], pe[:, lo:hi],
                            mask_sb[:, kt, lo:hi], mybir.AluOpType.mult,
                        )
                        pt[(h, kt)] = pm

            def emit_pv(hp):
                et = hp
                for sub in (0, 64):
                    h = 2 * hp + (sub // 64)
                    # PV banded: query tile t consumes k-tiles t..t+2;
                    # row 64 of the output collects sum(P) via the ones col
                    cxp = cxps.tile([128, 512], F32, tag="cx")
                    for t in range(NQT):
                        for kt in range(t, t + 3):
                            nc.tensor.matmul(
                                cxp[0:65, t * 128 : (t + 1) * 128],
                                v_sb[kt][:, h * 65 : (h + 1) * 65],
                                pt[(h, kt)][:, t * 128 : (t + 1) * 128],
                                start=(kt == t), stop=(kt == t + 2),
                            )
                    lr = attpool.tile([1, SL], F32, tag="lr")
                    nc.vector.tensor_copy(lr[:], cxp[64:65, :])
                    linv = attpool.tile([1, SL], F32, tag="linv")
                    nc.vector.reciprocal_approx_fast(linv[:], lr[:])
                    lbc = attpool.tile([64, SL], F32, tag="lbc")
                    nc.gpsimd.partition_broadcast(lbc[:], linv[:])
                    nc.vector.tensor_mul(ctx_sb[et][sub : sub + 64, :],
                                         cxp[0:64, :], lbc[:])

            # issue scores(hp) ahead of PV(hp-1): PV waits on exp+mask, so
            # keep independent scores matmuls in front of it in PE order
            for hp in range(H // 2):
                emit_scores(hp)
                if hp > 0:
                    emit_pv(hp - 1)
            emit_pv(H // 2 - 1)

            # ---------- output projection ----------
            wo_sb = []
            for kt in range(NET):
                t = wpool.tile([128, E], BF16, tag=f"wo{kt}")
                nc.sync.dma_start(t[:], wo[kt * 128 : (kt + 1) * 128, :])
                wo_sb.append(t)
            for eo in range(NET):
                op = mmps.tile([128, 512], F32, tag="mm")
                for et in range(NET):
                    nc.tensor.matmul(
                        op[:],
                        wo_sb[et][:, eo * 128 : (eo + 1) * 128],
                        ctx_sb[et][:],
                        start=(et == 0),
                        stop=(et == NET - 1),
                    )
                o_sb = outpool.tile([128, SL], F32, tag="o")
                nc.scalar.activation(
                    o_sb[:], op[:], mybir.ActivationFunctionType.Identity,
                    bias=bo_sb[:, eo : eo + 1], scale=1.0,
                )
                nc.sync.dma_start(out_ext[eo * 128 : (eo + 1) * 128, :], o_sb[:])

    nc.compile()
    return nc


_NC_CACHE = None
LAST_RESULT = None


def _get_graph():
    global _NC_CACHE
    if _NC_CACHE is None:
        _NC_CACHE = _build_graph()
    return _NC_CACHE


def _rot_matrix():
    # rot(q)[d] = -q[d+32] (d<32) ; q[d-32] (d>=32), per 64-block; 2 blocks.
    r64 = np.zeros((64, 64), dtype=np.float32)
    for d in range(32):
        r64[d, d + 32] = -1.0
        r64[d + 32, d] = 1.0
    r = np.zeros((128, 128), dtype=np.float32)
    r[0:64, 0:64] = r64
    r[64:128, 64:128] = r64
    return r


def kernel(x, mask, cos, sin, Wq, bq, Wk, Wv, bv, Wo, bo):
    x = np.asarray(x, dtype=np.float32)
    cos = np.asarray(cos, dtype=np.float32)
    sin = np.asarray(sin, dtype=np.float32)
    B = x.shape[0]
    assert (B, S, E) == x.shape

    bf = lambda a: np.ascontiguousarray(a).astype(ml_dtypes.bfloat16)
    wq_b, wk_b, wv_b, wo_b = bf(Wq), bf(Wk), bf(Wv), bf(Wo)
    rt_b = bf(_rot_matrix().T)
    bq_t = np.ascontiguousarray(
        np.asarray(bq, np.float32).reshape(NET, 128).T)
    bo_t = np.ascontiguousarray(
        np.asarray(bo, np.float32).reshape(NET, 128).T)
    bv_t = np.ascontiguousarray(
        np.tile(np.asarray(bv, np.float32)[None, :], (128, 1)))

    # transposed band mask per k-tile: allowed iff q <= k_pad <= q + 256
    ki = np.arange(128)[:, None]
    qj = np.arange(SL)[None, :]

    in_maps = []
    for c in range(NCORES):
        lo = c * SL - HALO
        xp = np.zeros((SK, E), dtype=np.float32)
        cp = np.zeros((SK, D), dtype=np.float32)
        sp = np.zeros((SK, D), dtype=np.float32)
        src_lo = max(lo, 0)
        dst_lo = src_lo - lo
        xp[dst_lo:] = x[0, src_lo : lo + SK]
        cp[dst_lo:] = cos[0, src_lo : lo + SK]
        sp[dst_lo:] = sin[0, src_lo : lo + SK]
        m = np.zeros((128, NKT, SL), dtype=np.float32)
        for kt in range(NKT):
            k_pad = kt * 128 + ki
            valid = (qj <= k_pad) & (k_pad <= qj + HALO)
            if c == 0:
                valid &= k_pad >= HALO
            m[:, kt, :] = valid.astype(np.float32)
        m = m.astype(ml_dtypes.bfloat16)
        in_maps.append({
            "xT": bf(xp.T),
            "wq": wq_b, "wk": wk_b, "wv": wv_b, "wo": wo_b,
            "rt": rt_b,
            "bq": bq_t, "bo": bo_t, "bv": bv_t,
            "cosT": np.ascontiguousarray(np.tile(cp.T, (2, 1))),
            "sinT": np.ascontiguousarray(np.tile(sp.T, (2, 1))),
            "maskadd": m,
        })

    nc = _get_graph()
    trace = bool(os.environ.get("BASS_KERNEL_TRACE"))
    if trace:
        _ensure_ntff_hook()
    res = run_bass_kernel_spmd(
        nc, in_maps, core_ids=list(range(NCORES)), trace=trace
    )
    global LAST_RESULT
    LAST_RESULT = res

    out = np.empty((1, S, E), dtype=np.float32)
    for c in range(NCORES):
        out[0, c * SL : (c + 1) * SL, :] = res.results[c]["out"].T
    return out


if __name__ == "__main__":
    import reference
    inputs = reference.setup_inputs()
    inputs = {k: np.asarray(v) for k, v in inputs.items()}
    got = kernel(**inputs)
    exp = np.asarray(reference.reference(**inputs))
    err = np.abs(got - exp).max() / np.abs(exp).max()
    print("rel err:", err)

